# revision 30
# baseline (speedup 1.0000x reference)
"""Multi-head attention TRN2 Bass kernel (B=4, S=2048, E=2048, H=16, D=128).

Sharding: 2 heads per core (tensor parallel over H=16 across 8 cores).
Each core computes q/k/v projections for its 2 heads over all batches,
attention, and a partial out-projection (its heads' columns of W_out).
Host sums the 8 fp32 partial outputs (the "all-reduce") and transposes.

Device layouts (per core):
  xt   [B, E, S]  bf16   x transposed per batch (feature-major)
  wqk  [E, 4D]    bf16   W_q/W_k columns for heads (q0|q1|k0|k1)
  wv   [E, 2D]    bf16   W_v columns (v0|v1)
  wo   [2D, E]    bf16   W_out^T rows for this core's head channels
  yt   [B, E, S]  f32    partial output, feature-major (ExternalOutput)
"""

import os
import sys

sys.path.insert(0, "/opt/trn_rl_repo")

import numpy as np
import ml_dtypes

B, S, E = 4, 2048, 2048
H, D = 16, 128
NCORES = 8
HPC = H // NCORES  # 2 heads per core
SCALE = 1.0 / float(np.sqrt(D))

EC = E // 128     # 16 contraction chunks
TCX = 1024        # x sbuf tile width (tokens)
KC = S // 128     # 16 key chunks
NQP = S // 1024   # 2 query chunk-pairs


def _build():
    import concourse.bass as bass
    import concourse.tile as tile
    from concourse import bacc, mybir
    from concourse import bass_isa

    bf = mybir.dt.bfloat16
    f16 = mybir.dt.float16
    f32 = mybir.dt.float32
    ADD = mybir.AluOpType.add
    MULT = mybir.AluOpType.mult
    EXP = mybir.ActivationFunctionType.Exp

    nc = bacc.Bacc(
        "TRN2", target_bir_lowering=False, debug=False, num_devices=NCORES
    )
    xt = nc.dram_tensor("xt", [B, E, S], bf, kind="ExternalInput").ap()
    wqk = nc.dram_tensor("wqk", [E, 4 * D], bf, kind="ExternalInput").ap()
    wv = nc.dram_tensor("wv", [E, 2 * D], bf, kind="ExternalInput").ap()
    wo = nc.dram_tensor("wo", [2 * D, E], bf, kind="ExternalInput").ap()
    yt = nc.dram_tensor("yt", [B, E, S], f16, kind="ExternalOutput").ap()

    with tile.TileContext(nc) as tc:
        with (
            tc.tile_pool(name="wp", bufs=1) as wp,
            tc.tile_pool(name="xp", bufs=24) as xp,
            tc.tile_pool(name="qkp", bufs=8) as qkp,
            tc.tile_pool(name="vp", bufs=32) as vp,
            tc.tile_pool(name="op", bufs=4) as osp,
            tc.tile_pool(name="ptp", bufs=8) as ptp,
            tc.tile_pool(name="accp", bufs=3) as accp,
            tc.tile_pool(name="dbp", bufs=3) as dbp,
            tc.tile_pool(name="yp", bufs=4) as yp,
            tc.tile_pool(name="oup", bufs=6) as oup,
            tc.tile_pool(name="psp", bufs=2, space="PSUM") as psp,
        ):
            # ---- load weights (resident) ----
            wqk_sb = wp.tile([128, EC * 4 * D], bf, tag="wqk", name="wqk_sb")
            for ec in range(EC):
                nc.sync.dma_start(
                    wqk_sb[:, ec * 512 : (ec + 1) * 512],
                    wqk[ec * 128 : (ec + 1) * 128, :],
                )
            # wv/wo loads are deferred until after batch 0's first x tiles so
            # the startup DMA critical path is just wqk + x(tcx0).
            wv_sb = wp.tile([128, EC * 2 * D], bf, tag="wv", name="wv_sb")
            wo_sb = wp.tile([128, 2 * E], bf, tag="wo", name="wo_sb")
            ones_sb = wp.tile([128, 1], f16, tag="ones", name="ones_sb")
            nc.vector.memset(ones_sb, 1.0)

            def load_wv():
                for ec in range(EC):
                    nc.sync.dma_start(
                        wv_sb[:, ec * 256 : (ec + 1) * 256],
                        wv[ec * 128 : (ec + 1) * 128, :],
                    )

            def load_wo():
                for cc in range(2):
                    nc.sync.dma_start(
                        wo_sb[:, cc * E : (cc + 1) * E],
                        wo[cc * 128 : (cc + 1) * 128, :],
                    )

            for b in range(B):
                # ================= QKV projection for batch b =================
                qk_tiles = [
                    qkp.tile([128, S], bf, tag="qk", name=f"qk_{b}_{mc}")
                    for mc in range(4)  # q_h0, q_h1, k_h0, k_h1
                ]
                v_tiles = [
                    vp.tile([128, 2 * D], f16, tag="v", name=f"v_{b}_{kc}")
                    for kc in range(KC)
                ]
                for tcx in range(S // TCX):
                    x_tiles = []
                    for ec in range(EC):
                        xtile = xp.tile(
                            [128, TCX], bf, tag="x", name=f"x_{b}_{tcx}_{ec}"
                        )
                        nc.sync.dma_start(
                            xtile,
                            xt[
                                b,
                                ec * 128 : (ec + 1) * 128,
                                tcx * TCX : (tcx + 1) * TCX,
                            ],
                        )
                        x_tiles.append(xtile)
                    if b == 0 and tcx == 0:
                        load_wv()
                    # Q/K: out [D, tokens] per head; lhsT = w chunk, rhs = x chunk
                    for mc in range(4):
                        for tcs in range(2):
                            pq = psp.tile(
                                [128, 512], f32, tag="pq", name=f"pq_{b}_{tcx}_{mc}_{tcs}"
                            )
                            for ec in range(EC):
                                nc.tensor.matmul(
                                    pq,
                                    lhsT=wqk_sb[
                                        :, ec * 512 + mc * 128 : ec * 512 + (mc + 1) * 128
                                    ],
                                    rhs=x_tiles[ec][:, tcs * 512 : (tcs + 1) * 512],
                                    start=(ec == 0),
                                    stop=(ec == EC - 1),
                                )
                            tchunk = tcx * TCX + tcs * 512
                            nc.vector.tensor_copy(
                                qk_tiles[mc][:, tchunk : tchunk + 512], pq
                            )
                    # V: out [tokens, 2D]; lhsT = x chunk (tokens cols), rhs = wv
                    for tsub in range(TCX // 128):
                        pv = psp.tile(
                            [128, 2 * D], f32, tag="pq", name=f"pv_{b}_{tcx}_{tsub}"
                        )
                        for ec in range(EC):
                            nc.tensor.matmul(
                                pv,
                                lhsT=x_tiles[ec][:, tsub * 128 : (tsub + 1) * 128],
                                rhs=wv_sb[:, ec * 256 : (ec + 1) * 256],
                                start=(ec == 0),
                                stop=(ec == EC - 1),
                            )
                        kc = tcx * (TCX // 128) + tsub
                        nc.vector.tensor_copy(v_tiles[kc], pv)

                # ================= attention for batch b =================
                if b == 0:
                    load_wo()
                out_sb = [
                    osp.tile([128, S], bf, tag="osb", name=f"osb_{b}_{h}")
                    for h in range(HPC)
                ]
                for qp in range(NQP):
                    for h in range(HPC):
                        q_t = qk_tiles[h]       # [D, S]
                        k_t = qk_tiles[2 + h]   # [D, S]
                        q0 = qp * 1024
                        out_a = psp.tile(
                            [128, 512], f32, tag="oc", name=f"oa_{b}_{h}_{qp}"
                        )
                        out_b = psp.tile(
                            [128, 512], f32, tag="oc", name=f"ob_{b}_{h}_{qp}"
                        )
                        acc = accp.tile(
                            [128, 1024], f16, tag="acc", name=f"acc_{b}_{h}_{qp}"
                        )
                        for kc in range(KC):
                            sps = psp.tile(
                                [128, 1024], f32, tag="sc", name=f"s_{b}_{h}_{qp}_{kc}"
                            )
                            nc.tensor.matmul(
                                sps[:, :512],
                                lhsT=k_t[:, kc * 128 : (kc + 1) * 128],
                                rhs=q_t[:, q0 : q0 + 512],
                                start=True,
                                stop=True,
                            )
                            nc.tensor.matmul(
                                sps[:, 512:],
                                lhsT=k_t[:, kc * 128 : (kc + 1) * 128],
                                rhs=q_t[:, q0 + 512 : q0 + 1024],
                                start=True,
                                stop=True,
                            )
                            pt = ptp.tile(
                                [128, 1024], f16, tag="pt", name=f"pt_{b}_{h}_{qp}_{kc}"
                            )
                            nc.scalar.activation(pt, sps, EXP, scale=SCALE)
                            vt = v_tiles[kc]
                            nc.tensor.matmul(
                                out_a,
                                lhsT=vt[:, h * 128 : (h + 1) * 128],
                                rhs=pt[:, :512],
                                start=(kc == 0),
                                stop=(kc == KC - 1),
                            )
                            nc.tensor.matmul(
                                out_b,
                                lhsT=vt[:, h * 128 : (h + 1) * 128],
                                rhs=pt[:, 512:],
                                start=(kc == 0),
                                stop=(kc == KC - 1),
                            )
                            if kc == 1:
                                nc.vector.tensor_tensor(acc, pt_prev, pt, ADD)
                            elif kc > 1:
                                nc.vector.tensor_tensor(acc, acc, pt, ADD)
                            pt_prev = pt
                        # Drain attnV accumulators to SBUF immediately: frees the
                        # "oc" PSUM slots so the next chunk's attnV is not blocked
                        # behind the allreduce+reciprocal tail.
                        ou_a = oup.tile(
                            [128, 512], f16, tag="ou", name=f"oua_{b}_{h}_{qp}"
                        )
                        ou_b = oup.tile(
                            [128, 512], f16, tag="ou", name=f"oub_{b}_{h}_{qp}"
                        )
                        nc.any.tensor_copy(ou_a, out_a)
                        nc.any.tensor_copy(ou_b, out_b)
                        # denominator: partition-sum of acc via ones-matmul (PE),
                        # fast reciprocal on the [1, 1024] row, then GPSIMD
                        # broadcast to all partitions for the normalizing mult.
                        rec_sb = dbp.tile(
                            [1, 1024], f32, tag="rec", name=f"rec_{b}_{h}_{qp}"
                        )
                        for dh in range(2):
                            den_ps = psp.tile(
                                [1, 512], f32, tag="pq", name=f"den_{b}_{h}_{qp}_{dh}"
                            )
                            nc.tensor.matmul(
                                den_ps,
                                lhsT=ones_sb,
                                rhs=acc[:, dh * 512 : (dh + 1) * 512],
                                start=True,
                                stop=True,
                            )
                            nc.vector.reciprocal_approx_fast(
                                out=rec_sb[:, dh * 512 : (dh + 1) * 512], in_=den_ps
                            )
                        dbc = dbp.tile(
                            [128, 1024], f32, tag="dbc", name=f"dbc_{b}_{h}_{qp}"
                        )
                        nc.gpsimd.partition_broadcast(dbc, rec_sb)
                        nc.vector.tensor_tensor(
                            out_sb[h][:, q0 : q0 + 512], ou_a, dbc[:, :512], MULT
                        )
                        nc.vector.tensor_tensor(
                            out_sb[h][:, q0 + 512 : q0 + 1024], ou_b, dbc[:, 512:], MULT
                        )

                # ================= out-projection for batch b =================
                for tp in range(S // 1024):
                    for fc in range(E // 128):
                        yps = psp.tile(
                            [128, 1024], f32, tag="sc", name=f"y_{b}_{fc}_{tp}"
                        )
                        for half in range(2):
                            tok = tp * 1024 + half * 512
                            for cc in range(HPC):
                                nc.tensor.matmul(
                                    yps[:, half * 512 : (half + 1) * 512],
                                    lhsT=wo_sb[
                                        :, cc * E + fc * 128 : cc * E + (fc + 1) * 128
                                    ],
                                    rhs=out_sb[cc][:, tok : tok + 512],
                                    start=(cc == 0),
                                    stop=(cc == HPC - 1),
                                )
                        y_sb = yp.tile(
                            [128, 1024], f16, tag="y", name=f"ysb_{b}_{fc}_{tp}"
                        )
                        if fc % 2 == 0:
                            nc.scalar.copy(y_sb, yps)
                        else:
                            nc.vector.tensor_copy(y_sb, yps)
                        nc.sync.dma_start(
                            yt[b, fc * 128 : (fc + 1) * 128, tp * 1024 : (tp + 1) * 1024],
                            y_sb,
                        )
    nc.compile()
    return nc


_NC_CACHE = None
LAST_EXEC_NS = None


def _ensure_trace_hook_stub():
    """If the image's antenv lacks axon_hooks, a stray BASS_TRACE env var
    would crash run_bass_kernel_spmd on import. Register a None-hook stub
    (concourse then logs a warning and runs without tracing)."""
    try:
        import antenv.axon_hooks  # noqa: F401
    except ImportError:
        import types

        mod = types.ModuleType("antenv.axon_hooks")
        mod.get_axon_ntff_profile_hook = lambda: None
        mod.set_axon_ntff_profile_hook = lambda h: None
        sys.modules["antenv.axon_hooks"] = mod


def kernel(**inputs):
    global _NC_CACHE, LAST_EXEC_NS
    _ensure_trace_hook_stub()
    from concourse import bass_utils

    x = np.asarray(inputs["x"], dtype=np.float32)
    w_qkv = np.asarray(inputs["w_qkv"], dtype=np.float32)
    w_out = np.asarray(inputs["w_out"], dtype=np.float32)

    bf = ml_dtypes.bfloat16
    xt_np = np.ascontiguousarray(x.transpose(0, 2, 1)).astype(bf)  # [B, E, S]

    in_maps = []
    for c in range(NCORES):
        rows_q, rows_k, rows_v = [], [], []
        for h in (HPC * c, HPC * c + 1):
            base = h * 3 * D
            rows_q.append(w_qkv[base : base + D])
            rows_k.append(w_qkv[base + D : base + 2 * D])
            rows_v.append(w_qkv[base + 2 * D : base + 3 * D])
        wqk_c = np.concatenate(rows_q + rows_k, axis=0).T  # [E, 4D]
        wv_c = np.concatenate(rows_v, axis=0).T            # [E, 2D]
        wo_c = w_out[:, HPC * c * D : (HPC * c + HPC) * D].T  # [2D, E]
        in_maps.append(
            {
                "xt": xt_np,
                "wqk": np.ascontiguousarray(wqk_c).astype(bf),
                "wv": np.ascontiguousarray(wv_c).astype(bf),
                "wo": np.ascontiguousarray(wo_c).astype(bf),
            }
        )

    if _NC_CACHE is None:
        _NC_CACHE = _build()
    nc = _NC_CACHE

    res = bass_utils.run_bass_kernel_spmd(
        nc, in_maps, core_ids=list(range(NCORES))
    )
    LAST_EXEC_NS = res.exec_time_ns

    y_t = res.results[0]["yt"].astype(np.float32)
    for c in range(1, NCORES):
        y_t += res.results[c]["yt"].astype(np.float32)
    return np.ascontiguousarray(y_t.transpose(0, 2, 1)).astype(np.float32)
